# revision 5
# baseline (speedup 1.0000x reference)
"""Trainium2 Bass kernel for causal multi-head attention (dense transformer block).

Math (reference semantics):
    qkv = x @ w_qkv.T ; split into Q,K,V heads [B,H,T,dk]
    (rotary in the reference rotates Q and K of head h by a constant,
     time-independent orthogonal rotation R_h; since scores = (R_h q)·(R_h k)
     = q·k, the rotation cancels exactly and is skipped here)
    scores = causal_mask(Q @ K.T / sqrt(dk)); attn = softmax(scores)
    out = attn @ V ; y = out @ w_o.T

Sharding: head-parallel over 8 cores (2 heads/core, both batches).  Each core
computes a partial y (its heads' contribution through w_o columns); the host
sums the 8 partials (the "all-reduce").

All matmuls run as float32r (TF32-class precision, full PE rate at N>=256).
"""

import numpy as np

import concourse.bacc as bacc
import concourse.bass as bass
import concourse.mybir as mybir
import concourse.tile as tile
from concourse import bass_utils
from concourse.masks import make_identity

B, T, D, H, DK = 2, 2048, 2048, 16, 128
NCORES = 8
HPC = H // NCORES  # heads per core
P = 128
NB = 512  # free-dim block (phase1 token block, phase2 query block, phase3 e block)
KC = D // P  # 16 contraction chunks of the model dim
QB = T // NB  # 4 query blocks per batch
NT = T // P  # 16 token tiles / key tiles per batch
FP32 = mybir.dt.float32
F32R = mybir.dt.float32r
SCALE = 1.0 / np.sqrt(DK)


def build(debug=False):
    nc = bacc.Bacc("TRN2", target_bir_lowering=False, debug=False, num_devices=NCORES)
    xT = nc.dram_tensor("xT", [D, B * T], F32R, kind="ExternalInput")
    wqkvT = nc.dram_tensor("wqkvT", [D, 6 * DK], F32R, kind="ExternalInput")
    woT = nc.dram_tensor("woT", [HPC * DK, D], F32R, kind="ExternalInput")
    masks = nc.dram_tensor("masks", [4, P, NB], F32R, kind="ExternalInput")
    ones_col = nc.dram_tensor("ones_col", [P, 1], F32R, kind="ExternalInput")
    ones_row = nc.dram_tensor("ones_row", [1, P], F32R, kind="ExternalInput")
    y = nc.dram_tensor("y", [B * T, D], FP32, kind="ExternalOutput")
    dbg = {}
    if debug:
        dbg["qkvT"] = nc.dram_tensor("dbg_qkvT", [6 * DK, B * T], F32R, kind="ExternalOutput")
        dbg["outT"] = nc.dram_tensor("dbg_outT", [HPC * DK, B * T], F32R, kind="ExternalOutput")

    with tile.TileContext(nc) as tc:
        with (
            tc.tile_pool(name="const", bufs=1) as cpool,
            tc.tile_pool(name="xp", bufs=2) as xpool,
            tc.tile_pool(name="qkv", bufs=1) as qpool,
            tc.tile_pool(name="attn", bufs=1) as apool,
            tc.tile_pool(name="ps", bufs=1, space="PSUM") as pspool,
        ):
            # ---- constants / weights resident in SBUF ----
            w_sb = [cpool.tile([P, 6 * DK], F32R, name=f"w_{k}") for k in range(KC)]
            for k in range(KC):
                nc.sync.dma_start(w_sb[k][:], wqkvT[k * P:(k + 1) * P, :])
            wo_sb = [cpool.tile([P, D], F32R, name=f"wo_{h}") for h in range(HPC)]
            for h in range(HPC):
                nc.sync.dma_start(wo_sb[h][:], woT[h * P:(h + 1) * P, :])
            mask_sb = [cpool.tile([P, NB], F32R, name=f"mask_{j}") for j in range(4)]
            for j in range(4):
                nc.sync.dma_start(mask_sb[j][:], masks[j])
            onc = cpool.tile([P, 1], F32R, name="onc")
            nc.sync.dma_start(onc[:], ones_col[:, :])
            onr = cpool.tile([1, P], F32R, name="onr")
            nc.sync.dma_start(onr[:], ones_row[:, :])
            ident = cpool.tile([P, P], FP32, name="ident")
            make_identity(nc, ident[:])

            # alternate PSUM tags so back-to-back groups double-buffer across
            # the two "spare" tags without exceeding the 8-bank budget
            def ps_alt(i, shape, name):
                return pspool.tile(shape, FP32, name=name,
                                   tag="ps_y" if i % 2 == 0 else "ps_o", bufs=2)

            for b in range(B):
                # per-batch tiles; tags make slots recycle across batches.
                # VT shares slots with outT (disjoint lifetimes within a batch).
                QT = [qpool.tile([P, T], F32R, name=f"QT{h}_{b}", tag=f"QT{h}") for h in range(HPC)]
                KT = [qpool.tile([P, T], F32R, name=f"KT{h}_{b}", tag=f"KT{h}") for h in range(HPC)]
                VT = [qpool.tile([P, T], FP32, name=f"VT{h}_{b}", tag=f"vo{h}") for h in range(HPC)]
                V = [[qpool.tile([P, DK], F32R, name=f"V{h}_{kt}_{b}", tag=f"V{h}_{kt}")
                      for kt in range(NT)] for h in range(HPC)]
                mdest = [QT[0], KT[0], VT[0], QT[1], KT[1], VT[1]]

                # ======== phase 1: QKV^T = wqkvT.T @ xT_block ========
                for nb in range(QB):
                    xt = [xpool.tile([P, NB], F32R, name=f"x{k}_{b}_{nb}", tag=f"x{k}",
                                     bufs=(2 if k < 11 else 1))
                          for k in range(KC)]
                    col0 = b * T + nb * NB
                    for k in range(KC):
                        nc.sync.dma_start(xt[k][:], xT[k * P:(k + 1) * P, col0:col0 + NB])
                    for m in range(6):
                        ps = pspool.tile([P, NB], FP32, name=f"ps1_{b}_{nb}_{m}",
                                         tag="ps_s", bufs=3)
                        for k in range(KC):
                            nc.tensor.matmul(ps[:], w_sb[k][:, m * P:(m + 1) * P],
                                             xt[k][:], start=(k == 0), stop=(k == KC - 1))
                        nc.scalar.copy(mdest[m][:, nb * NB:(nb + 1) * NB], ps[:])
                # V^T -> V (PE transpose per 128x128 tile)
                for h in range(HPC):
                    for kt in range(NT):
                        pst = ps_alt(kt, [P, P], f"pst_{b}_{h}_{kt}")
                        nc.tensor.transpose(pst[:], VT[h][:, kt * P:(kt + 1) * P], ident[:])
                        nc.vector.tensor_copy(V[h][kt][:], pst[:])

                if debug:
                    for m in range(6):
                        nc.sync.dma_start(
                            dbg["qkvT"][m * P:(m + 1) * P, b * T:(b + 1) * T], mdest[m][:])

                # ======== phase 2: attention per head ========
                outT = [qpool.tile([P, T], F32R, name=f"outT{h}_{b}", tag=f"vo{h}")
                        for h in range(HPC)]
                for h in range(HPC):
                    for qb in range(QB):
                        nkt = 4 * qb + 4  # causal: key tiles 0..nkt-1
                        ps_o = pspool.tile([P, NB], FP32, name=f"pso_{b}_{h}_{qb}",
                                           tag="ps_o", bufs=2)
                        ps_d = pspool.tile([1, NB], FP32, name=f"psd_{b}_{h}_{qb}",
                                           tag="ps_d", bufs=1)
                        qsl = slice(qb * NB, (qb + 1) * NB)

                        def scores(kt, b=b, h=h, qb=qb, qsl=qsl):
                            ps_s = pspool.tile([P, NB], FP32, name=f"pss_{b}_{h}_{qb}_{kt}",
                                               tag="ps_s", bufs=3)
                            nc.tensor.matmul(ps_s[:], KT[h][:, kt * P:(kt + 1) * P],
                                             QT[h][:, qsl], start=True, stop=True)
                            return ps_s

                        pss = {0: scores(0)}
                        if nkt > 1:
                            pss[1] = scores(1)
                        for kt in range(nkt):
                            ps_s = pss.pop(kt)
                            ex = apool.tile([P, NB], F32R, name=f"ex_{b}_{h}_{qb}_{kt}",
                                            tag="ex", bufs=4)
                            nc.scalar.activation(ex[:], ps_s[:],
                                                 mybir.ActivationFunctionType.Exp,
                                                 scale=SCALE)
                            j = kt - 4 * qb
                            if j >= 0:
                                nc.vector.tensor_mul(ex[:], ex[:], mask_sb[j][:])
                            if kt + 2 < nkt:
                                pss[kt + 2] = scores(kt + 2)
                            nc.tensor.matmul(ps_d[:1, :], onc[:, :], ex[:],
                                             start=(kt == 0), stop=(kt == nkt - 1),
                                             skip_group_check=True)
                            nc.tensor.matmul(ps_o[:], V[h][kt][:], ex[:],
                                             start=(kt == 0), stop=(kt == nkt - 1),
                                             skip_group_check=True)
                        # normalize: outT[:, qsl] = ps_o * (1/ps_d) broadcast over partitions
                        rec = apool.tile([1, NB], F32R, name=f"rec_{b}_{h}_{qb}",
                                         tag="rec", bufs=1)
                        with nc.allow_low_precision(reason="f32r recip: tf32 rounding intended"):
                            nc.vector.reciprocal(rec[:1, :], ps_d[:1, :])
                        ps_b = pspool.tile([P, NB], FP32, name=f"psb_{b}_{h}_{qb}",
                                           tag="ps_s", bufs=3)
                        nc.tensor.matmul(ps_b[:], onr[:1, :], rec[:1, :],
                                         start=True, stop=True)
                        bc = apool.tile([P, NB], FP32, name=f"bc_{b}_{h}_{qb}",
                                        tag="bc", bufs=1)
                        nc.scalar.copy(bc[:], ps_b[:])
                        nc.vector.tensor_mul(outT[h][:, qsl], ps_o[:], bc[:])
                if debug:
                    for h in range(HPC):
                        nc.sync.dma_start(
                            dbg["outT"][h * P:(h + 1) * P, b * T:(b + 1) * T], outT[h][:])

                # ======== phase 3: y_partial = outT.T @ woT ========
                for tt in range(NT):
                    for eb in range(QB):
                        ps = ps_alt(tt * QB + eb, [P, NB], f"psy_{b}_{tt}_{eb}")
                        for h in range(HPC):
                            nc.tensor.matmul(ps[:], outT[h][:, tt * P:(tt + 1) * P],
                                             wo_sb[h][:, eb * NB:(eb + 1) * NB],
                                             start=(h == 0), stop=(h == HPC - 1))
                        yt = apool.tile([P, NB], FP32, name=f"yt_{b}_{tt}_{eb}",
                                        tag="yt", bufs=2)
                        nc.any.tensor_copy(yt[:], ps[:])
                        nc.sync.dma_start(
                            y[b * T + tt * P: b * T + (tt + 1) * P, eb * NB:(eb + 1) * NB],
                            yt[:])

    nc.compile()
    return nc


def prep_inputs(x, w_qkv, w_o):
    """Host-side shard prep. Returns per-core input maps."""
    x = np.ascontiguousarray(np.asarray(x, dtype=np.float32).reshape(B * T, D).T)
    w_qkv = np.asarray(w_qkv, dtype=np.float32)
    w_o = np.asarray(w_o, dtype=np.float32)

    mask = np.zeros((4, P, NB), dtype=np.float32)
    for j in range(4):
        kp = np.arange(P)[:, None] + j * P
        qf = np.arange(NB)[None, :]
        mask[j] = (kp <= qf).astype(np.float32)

    ones_col = np.ones((P, 1), dtype=np.float32)
    ones_row = np.ones((1, P), dtype=np.float32)

    in_maps = []
    for c in range(NCORES):
        h0, h1 = HPC * c, HPC * c + 1
        blocks = []
        for h in (h0, h1):
            blocks += [w_qkv[h * DK:(h + 1) * DK],          # Q rows
                       w_qkv[D + h * DK: D + (h + 1) * DK],  # K rows
                       w_qkv[2 * D + h * DK: 2 * D + (h + 1) * DK]]  # V rows
        # order: [Qh0,Kh0,Vh0,Qh1,Kh1,Vh1]
        wq = np.ascontiguousarray(np.concatenate(blocks, axis=0).T)  # [D, 768]
        wo = np.ascontiguousarray(w_o[:, HPC * DK * c: HPC * DK * (c + 1)].T)  # [256, D]
        in_maps.append({
            "xT": x, "wqkvT": wq, "woT": wo,
            "masks": mask, "ones_col": ones_col, "ones_row": ones_row,
        })
    return in_maps


_nc_cache = {}


def get_nc(debug=False):
    if debug not in _nc_cache:
        _nc_cache[debug] = build(debug=debug)
    return _nc_cache[debug]


def run(x, w_qkv, w_o, debug=False):
    nc = get_nc(debug=debug)
    in_maps = prep_inputs(x, w_qkv, w_o)
    res = bass_utils.run_bass_kernel_spmd(nc, in_maps, core_ids=list(range(NCORES)))
    return res


def kernel(x, w_qkv, w_o):
    res = run(x, w_qkv, w_o)
    y = res.results[0]["y"].astype(np.float64)
    for c in range(1, NCORES):
        y += res.results[c]["y"]
    return y.astype(np.float32).reshape(B, T, D)
